# revision 59
# baseline (speedup 1.0000x reference)
"""CorrelationHead Trainium2 kernel (v3: saturated-DMA schedule + slim sync).

Math: SpatialCorrelationSampler(patch=16, dil=2) on 7x7 maps zero-pads x2 by
(14,16). corr[b] (12544 features) has exactly 2401 distinct nonzero values =
Gram matrix G[b][kl,ij] = sum_c x1[b,c,ij]*x2[b,c,kl], and only parity-valid
(kl,ij) pairs (k=i mod 2, l=j mod 2) ever appear in corr: 625 of 2401.
fc1(corr) = sum over the 625 valid pairs of G * W1eff, so we:
  1. gram:  per-RoI G^T[kl, ij] on PE (K=256 contraction, 2 matmuls),
     evict to SBUF gsa[49, 128b, 49ij] (DVE), 16 chunks of 8 RoIs.
  2. stack: 49 selection matmuls per b-half (0/1 Sel, M=32-row slot, N=64)
     scatter the 625 valid (kl,ij) rows into 6 dense K<=128 "stacks" in PSUM.
     b1 folds in as a constant ones-row. Half 0 is spread over the gram
     stream (3 slots per chunk 8..15); half 1 interleaves with fc1.
  3. fc1:   6 stack-outer matmul pairs (hf inner, stationary reused) paced
     by the per-stack w1 DMAs.
  4. tail:  PE transpose (4-slot PSUM ring) + fc2 (k-outer, w2-DMA-paced)
     + transpose + fc3 in output-transposed form (psO[4,128b], N=128
     streams, 4-descriptor output DMA; host transposes back).

DMA: the two HWDGE queues carry x strictly first (evens on SP, odds on ACT;
~410 GB/s aggregate saturation), then w1 per-stack, then w2 per-k, so the
x stream is never starved by weight traffic. The gpsimd SWDGE carries only
the small constants (sel, biases, identity) up front. An early dummy Relu
pre-loads the ACT activation table off the critical path.

Sync: 6 counting semaphores total (two HWDGE queue counters, one SWDGE
counter, one per compute engine) with cumulative thresholds; HWDGE FIFO
order per engine slice makes >=16*(k+1) waits exact. Fewer semaphores
also shrink the sequencer teardown that is included in measured exec time.

Sharding: pure data-parallel over the 1024 RoIs -> 128 per each of 8 cores.
"""

import os
import numpy as np

import concourse.bass as bass
import concourse.mybir as mybir
from concourse.bass_utils import run_bass_kernel_spmd

# ---------------------------------------------------------------- constants
P = 16
DIL = 2
H = 7
C = 256
B = 1024
REP = 1024
HW = H * H  # 49
N_CORES = 8
BL = B // N_CORES  # 128 RoIs per core

NCHUNK = 8
CB = BL // NCHUNK        # 16 RoIs per chunk
NG = 8                   # RoIs per gram PSUM group (bank = 2KB)
GPC = CB // NG           # gram groups per chunk = 2
NGROUP = BL // NG        # 16
CHW = 2 * 2 * CB * HW    # els per partition per chunk = 3136

F32 = mybir.dt.float32
BF16 = mybir.dt.bfloat16

WARM = int(os.environ.get("CORR_WARM", "1"))

LAST_EXEC_NS = None
_CACHE = {}


# ------------------------------------------------------------- stack layout
def _klist(ij):
    i, j = ij // H, ij % H
    return [k * H + l for k in range(i % 2, H, 2) for l in range(j % 2, H, 2)]


def _layout():
    """Slot/stack assignment for the 625 valid (kl,ij) pairs.

    Returns slots: list of lists of ij; slot t lives at stack t//4,
    partition base 32*(t%4). Each ij occupies rows [off, off+nkl) of its
    slot where off = sum of nkl of earlier ijs in the slot.
    """
    EE = [i * H + j for i in range(0, H, 2) for j in range(0, H, 2)]  # 16x16
    EO = [i * H + j for i in range(0, H, 2) for j in range(1, H, 2)]  # 12x12
    OE = [i * H + j for i in range(1, H, 2) for j in range(0, H, 2)]  # 12x12
    OO = [i * H + j for i in range(1, H, 2) for j in range(1, H, 2)]  # 9x9
    slots = []
    slots += [[EE[2 * t], EE[2 * t + 1]] for t in range(8)]           # 32 rows
    slots += [[EO[2 * t], EO[2 * t + 1]] for t in range(6)]           # 24 rows
    slots += [[OE[2 * t], OE[2 * t + 1]] for t in range(6)]           # 24 rows
    slots += [[OO[3 * t], OO[3 * t + 1], OO[3 * t + 2]] for t in range(3)]
    assert len(slots) == 23
    return slots


SLOTS = _layout()
NSTACK = 6
STACK_K = [128, 128, 128, 128, 128, 97]  # stack 5: 3 slots + bias row at 96

# ------------------------------------------------- static sync thresholds
# s_pe increment positions (PE program order)
PE_GRAM = lambda gi: gi + 1                  # gram group gi evictable
PE_PERM1 = lambda s: 17 + s                  # perm half-1 stack s complete
PE_F10, PE_F11 = 23, 24                      # fc1 psF0 / psF1 done
PE_T1 = lambda k: 25 + k                     # transpose1 k in psT ring
PE_F20, PE_F21 = 33, 34                      # fc2 psF0 / psF1 done
PE_T2 = lambda k: 35 + k                     # transpose2 k in psT ring
PE_FC3 = 43                                  # fc3 psO done
# s_d increment positions (DVE program order)
D_GRAM = lambda gi: gi + 1                   # gsa group gi landed
D_STK = lambda s: 17 if s < 4 else 19        # Ssb stacks landed (bank copies)
D_CB1 = lambda k: 20 + k                     # r1T k landed
D_CB2 = lambda k: 28 + k                     # r2T k landed
# s_r increment positions (ACT program order)
R_RELU1 = lambda q: 1 + q
R_RELU2 = lambda q: 5 + q
R_OUT = 9
# DMA semaphores: increments from successive DMAs on a queue interleave
# across the 16 SDMA engine slices, so only the TOTAL over all DMAs on a
# semaphore is a valid wait. We pair each even-queue DMA with its odd-queue
# sibling on one semaphore (the pair lands together since the queues stream
# in parallel) and wait for 32.
WC_ALL = 16 * 8  # all gp const DMAs (tiny, land early; single total wait)


# ---------------------------------------------------------------- device IR
def _build():
    dt = BF16
    nc = bass.Bass()

    xh = nc.dram_tensor("xh", [128, NCHUNK * CHW], dt, kind="ExternalInput")
    selh = nc.dram_tensor("selh", [HW, HW * 32], dt, kind="ExternalInput")
    w1h = nc.dram_tensor("w1h", [128, NSTACK * REP], dt, kind="ExternalInput")
    w2h = nc.dram_tensor("w2h", [128, 8 * REP], dt, kind="ExternalInput")
    # fc3 M padded 4 -> 16 (zero cols) so psO[16, 128] is fully written and
    # the output DMA carries 16 partition descriptors (reliable +16 sem inc)
    w3h = nc.dram_tensor("w3h", [128, 8 * 16], dt, kind="ExternalInput")
    b2h = nc.dram_tensor("b2h", [1, REP], dt, kind="ExternalInput")
    b3h = nc.dram_tensor("b3h", [1, 16], dt, kind="ExternalInput")
    onesh = nc.dram_tensor("onesh", [1, 128], dt, kind="ExternalInput")
    identh = nc.dram_tensor("identh", [128, 128], dt, kind="ExternalInput")
    zbh = nc.dram_tensor("zbh", [128, 1], F32, kind="ExternalInput")
    # 16 partitions (rows 4:16 garbage): a HWDGE DMA only nets +16 on its
    # semaphore when all 16 SDMA engine slices carry descriptors
    outh = nc.dram_tensor("outh", [16, 128], F32, kind="ExternalOutput")

    from contextlib import ExitStack

    with ExitStack() as ctx:
        sb = lambda name, shape, d: ctx.enter_context(nc.sbuf_tensor(name, shape, d))
        ps = lambda name, shape, d: ctx.enter_context(nc.psum_tensor(name, shape, d))
        sem = lambda name: ctx.enter_context(nc.semaphore(name))

        xs = sb("xs", [128, NCHUNK, CHW], dt)
        gsa = sb("gsa", [HW, BL, HW], dt)
        sel = sb("sel", [HW, HW * 32], dt)
        w1s = sb("w1s", [128, NSTACK, REP], dt)
        w2s = sb("w2s", [128, 8 * REP], dt)
        w3s = sb("w3s", [128, 8 * 16], dt)
        Ssb = sb("Ssb", [128, NSTACK, 128], dt)
        b2s = sb("b2s", [1, REP], dt)
        b3s = sb("b3s", [1, 16], dt)
        ones = sb("ones", [1, 128], dt)
        idents = sb("idents", [128, 128], dt)
        zbias = sb("zbias", [128, 1], F32)
        dscr = sb("dscr", [128, 65], dt)  # dummy-relu + keepalive scratch
        relu1 = sb("relu1", [128, REP], dt)
        r1T = sb("r1T", [128, 8, 128], dt)
        relu2 = sb("relu2", [128, REP], dt)
        r2T = sb("r2T", [128, 8, 128], dt)
        outs = sb("outs", [16, 128], F32)

        psG0 = ps("psG0", [128, NG * HW], F32)      # 392 f32 = 1568B
        psG1 = ps("psG1", [128, NG * HW], F32)
        psS0 = ps("psS0", [128, 4, 128], F32)
        psS1 = ps("psS1", [128, 384], F32)  # stack4, stack5, psO region
        psF0 = ps("psF0", [128, 512], F32)
        psF1 = ps("psF1", [128, 512], F32)
        psT0 = ps("psT0", [128, 128], dt)
        psT1 = ps("psT1", [128, 128], dt)
        psO = psS1[0:16, 256:384]

        s_x = [sem(f"s_x{p}") for p in range(NCHUNK // 2)]   # chunk pairs
        s_w1 = sem("s_w1")                                   # both w1 halves
        s_w2 = [sem(f"s_w2{r}") for r in range(2)]           # w2 quad pairs
        s_wc = sem("s_wc")
        s_pe = sem("s_pe")
        s_d = sem("s_d")
        s_r = sem("s_r")
        s_o = sem("s_o")
        s_ka = sem("s_ka")  # keepalive junk (1-descriptor DMA, unreliable inc)

        block = ctx.enter_context(nc.Block())
        psG = [psG0, psG1]
        psF = [psF0, psF1]
        # 4-slot transpose ring: 2 dedicated bf16 banks + the (dead by then)
        # gram banks viewed as bf16
        psT = [
            psT0[:, :],
            psT1[:, :],
            psG0[:, 0:64].bitcast(BF16),
            psG1[:, 0:64].bitcast(BF16),
        ]

        # x slice: chunk layout per partition = [t, h, b, ij]
        def xsl(ch, t, h, b):
            off = ((t * 2 + h) * CB + b) * HW
            return xs[:, ch, off : off + HW]

        # ---------------- SP: even x chunks, w1 stacks 0-2, w2 k 0-1 / 4-5
        @block.sync
        def _(sp):
            for ch in range(0, NCHUNK, 2):
                sp.dma_start(
                    xs[:, ch, :], xh[:, ch * CHW : (ch + 1) * CHW]
                ).then_inc(s_x[ch // 2], 16)
            sp.dma_start(w1s[:, 0:3, :], w1h[:, 0 : 3 * REP]).then_inc(s_w1, 16)
            sp.dma_start(w2s[:, 0 : 2 * REP], w2h[:, 0 : 2 * REP]).then_inc(
                s_w2[0], 16
            )
            sp.dma_start(
                w2s[:, 4 * REP : 6 * REP], w2h[:, 4 * REP : 6 * REP]
            ).then_inc(s_w2[1], 16)
            sp.wait_ge(s_o, 16)

        # ---------------- GPSIMD: small constants via SWDGE
        @block.gpsimd
        def _(gp):
            gp.dma_start(zbias[:, :], zbh[:, :]).then_inc(s_wc, 16)
            gp.dma_start(sel[:, :], selh[:, :]).then_inc(s_wc, 16)
            gp.dma_start(ones[:, :], onesh[:, :]).then_inc(s_wc, 16)
            gp.dma_start(Ssb[96:97, 5, :], onesh[:, :]).then_inc(s_wc, 16)
            gp.dma_start(idents[:, :], identh[:, :]).then_inc(s_wc, 16)
            gp.dma_start(w3s[:, :], w3h[:, :]).then_inc(s_wc, 16)
            gp.dma_start(b2s[:, :], b2h[:, :]).then_inc(s_wc, 16)
            gp.dma_start(b3s[:, :], b3h[:, :]).then_inc(s_wc, 16)

        # ---------------- ACT: odd x chunks, w1 1/3/5, w2 k odd, relus, out
        @block.scalar
        def _(act):
            for ch in range(1, NCHUNK, 2):
                act.dma_start(
                    xs[:, ch, :], xh[:, ch * CHW : (ch + 1) * CHW]
                ).then_inc(s_x[ch // 2], 16)
            act.dma_start(w1s[:, 3:6, :], w1h[:, 3 * REP : 6 * REP]).then_inc(
                s_w1, 16
            )
            act.dma_start(
                w2s[:, 2 * REP : 4 * REP], w2h[:, 2 * REP : 4 * REP]
            ).then_inc(s_w2[0], 16)
            act.dma_start(
                w2s[:, 6 * REP : 8 * REP], w2h[:, 6 * REP : 8 * REP]
            ).then_inc(s_w2[1], 16)

            # pre-load the ACT activation table off the critical path
            act.wait_ge(s_wc, WC_ALL)
            act.activation(
                dscr[:, 0:1], zbias[:, :],
                mybir.ActivationFunctionType.Relu, bias=zbias[:, :],
            )

            for q in range(4):
                act.wait_ge(s_pe, PE_F10 if q < 2 else PE_F11)
                act.activation(
                    relu1[:, q * 256 : (q + 1) * 256],
                    psF[q // 2][:, (q % 2) * 256 : (q % 2) * 256 + 256],
                    mybir.ActivationFunctionType.Relu, bias=zbias[:, :],
                ).then_inc(s_r, 1)
            for q in range(4):
                act.wait_ge(s_pe, PE_F20 if q < 2 else PE_F21)
                act.activation(
                    relu2[:, q * 256 : (q + 1) * 256],
                    psF[q // 2][:, (q % 2) * 256 : (q % 2) * 256 + 256],
                    mybir.ActivationFunctionType.Relu, bias=zbias[:, :],
                ).then_inc(s_r, 1)
            # keepalive on the ACT HWDGE ring so the final output DMA does
            # not pay a cold-ring restart
            act.dma_start(dscr[0:1, 1:65], onesh[0:1, 0:64]).then_inc(s_ka, 16)
            act.wait_ge(s_pe, PE_FC3)
            act.activation(
                outs[:, :], psO, mybir.ActivationFunctionType.Copy
            ).then_inc(s_r, 1)
            act.wait_ge(s_r, R_OUT)
            act.dma_start(outh[:, :], outs[:, :]).then_inc(s_o, 16)

        # ---------------- PE: all matmuls
        @block.tensor
        def _(pe):
            # HAM warmer: a throwaway N<=512 matmul into psF0 (clobbered by
            # the next start=True accumulation) to keep the PE clock at 8/8
            def warm(n=1, w=512):
                if not WARM:
                    return
                for _ in range(n):
                    pe.matmul(
                        psF0[:, 0:w], xs[:, 0, 0:128], xs[:, 0, 0:w],
                        start=True, stop=True,
                    )

            # stack-scatter matmuls for slot t, batch columns c0:c1
            def perm_slot(t, c0, c1):
                ijs = SLOTS[t]
                st, base = t // 4, 32 * (t % 4)
                for u, ij in enumerate(ijs):
                    if st < 4:
                        pst = psS0[base : base + 32, st, c0:c1]
                    else:
                        pst = psS1[
                            base : base + 32,
                            (st - 4) * 128 + c0 : (st - 4) * 128 + c1,
                        ]
                    mm = pe.matmul(
                        pst,
                        sel[:, ij * 32 : (ij + 1) * 32],
                        gsa[:, c0:c1, ij],
                        start=(u == 0),
                        stop=(u == len(ijs) - 1),
                        tile_position=(0, base),
                    )
                return mm

            # fc1 for one stack: psF0/psF1 accumulate, stationary reused
            def fc1_stack(s):
                pe.wait_ge(s_w1, 32)
                pe.wait_ge(s_d, D_STK(s))
                ks = STACK_K[s]
                for hf in range(2):
                    mm = pe.matmul(
                        psF[hf][:, :],
                        Ssb[0:ks, s, :],
                        w1s[0:ks, s, hf * 512 : hf * 512 + 512],
                        start=(s == 0),
                        stop=(s == NSTACK - 1),
                    )
                    if s == NSTACK - 1:
                        mm.then_inc(s_pe, 1)  # PE_F10 then PE_F11

            # gram: G[b]^T[kl, ij] per RoI; perm half-0 spread over the tail
            for ch in range(NCHUNK):
                if ch % 2 == 0:
                    pe.wait_ge(s_x[ch // 2], 32)
                for g in range(GPC):
                    gi = ch * GPC + g
                    if gi >= 2:
                        pe.wait_ge(s_d, D_GRAM(gi - 2))
                    for bb in range(NG):
                        for h in range(2):
                            mm = pe.matmul(
                                psG[gi % 2][0:HW, bb * HW : (bb + 1) * HW],
                                xsl(ch, 1, h, g * NG + bb),
                                xsl(ch, 0, h, g * NG + bb),
                                start=(h == 0),
                                stop=(h == 1),
                            )
                    mm.then_inc(s_pe, 1)  # PE_GRAM(gi)
                    warm(1, 512)
                # perm half-0 spread over chunks 4..7 (x stream gaps)
                if ch >= 4:
                    if ch == 4:
                        pe.wait_ge(s_d, D_GRAM(7))
                        pe.wait_ge(s_wc, WC_ALL)
                    for t in range(6 * (ch - 4), min(6 * (ch - 4) + 6, 23)):
                        perm_slot(t, 0, 64)
                    warm(1, 512)

            # perm half-1, then fc1 (w1 lands after perm1 anyway)
            pe.wait_ge(s_d, D_GRAM(NGROUP - 1))
            for s in range(NSTACK):
                for t in range(4 * s, min(4 * s + 4, 23)):
                    mm = perm_slot(t, 64, 128)
                mm.then_inc(s_pe, 1)  # PE_PERM1(s)
                warm(1, 512)
            for s in range(NSTACK):
                fc1_stack(s)

            # transpose relu1 -> psT ring (DVE copies back)
            for k in range(8):
                pe.wait_ge(s_r, R_RELU1(k // 2))
                if k >= 4:
                    pe.wait_ge(s_d, D_CB1(k - 4))
                pe.transpose(
                    psT[k % 4], relu1[:, k * 128 : (k + 1) * 128], idents[:, :]
                ).then_inc(s_pe, 1)  # PE_T1(k)
                if k < 7:
                    warm(1, 256)

            # fc2: bias first (start=True), then 8 K-chunks, w2-DMA-paced,
            # stationary r1T[k] reused across the hf pair
            pe.wait_ge(s_r, R_RELU1(1))
            pe.matmul(psF0[:, :], ones[:, :], b2s[:, 0:512], start=True, stop=False)
            pe.wait_ge(s_r, R_RELU1(3))
            pe.matmul(psF1[:, :], ones[:, :], b2s[:, 512:1024], start=True, stop=False)
            for k in range(8):
                if k % 4 == 0:
                    pe.wait_ge(s_w2[k // 4], 32)
                pe.wait_ge(s_d, D_CB1(k))
                for hf in range(2):
                    mm = pe.matmul(
                        psF[hf][:, :],
                        r1T[:, k, :],
                        w2s[:, k * REP + hf * 512 : k * REP + hf * 512 + 512],
                        start=False,
                        stop=(k == 7),
                    )
                    if k == 7:
                        mm.then_inc(s_pe, 1)  # PE_F20 then PE_F21

            # transpose relu2 -> psT ring
            for k in range(8):
                pe.wait_ge(s_r, R_RELU2(k // 2))
                if k >= 4:
                    pe.wait_ge(s_d, D_CB2(k - 4))
                pe.transpose(
                    psT[k % 4], relu2[:, k * 128 : (k + 1) * 128], idents[:, :]
                ).then_inc(s_pe, 1)  # PE_T2(k)
                if k < 7:
                    warm(1, 256)

            # fc3 transposed: psO[16, 128b] = W3pad @ relu2^T + b3 (rows 4:16
            # are zero-padded so the 16-partition output DMA is fully defined)
            pe.matmul(psO, b3s[0:1, 0:16], ones[0:1, 0:128], start=True, stop=False)
            for k in range(8):
                pe.wait_ge(s_d, D_CB2(k))
                mm = pe.matmul(
                    psO,
                    w3s[:, k * 16 : (k + 1) * 16],
                    r2T[:, k, :],
                    start=False,
                    stop=(k == 7),
                )
            mm.then_inc(s_pe, 1)  # PE_FC3

        # ---------------- DVE: gram evicts, stack evicts, transpose copybacks
        @block.vector
        def _(dve):
            for gi in range(NGROUP):
                dve.wait_ge(s_pe, PE_GRAM(gi))
                dve.tensor_copy(
                    gsa[:, gi * NG : (gi + 1) * NG, :], psG[gi % 2][0:HW, :]
                ).then_inc(s_d, 1)  # D_GRAM(gi)
            # bank-coarse stack evicts (a read may not overlap any open
            # accumulation group in the same PSUM bank)
            dve.wait_ge(s_pe, PE_PERM1(3))
            dve.tensor_copy(Ssb[:, 0:4, :], psS0[:, :, :]).then_inc(s_d, 1)  # 17
            dve.wait_ge(s_pe, PE_PERM1(5))
            dve.tensor_copy(Ssb[:, 4, :], psS1[:, 0:128]).then_inc(s_d, 1)   # 18
            dve.tensor_copy(Ssb[0:96, 5, :], psS1[0:96, 128:256]).then_inc(
                s_d, 1
            )  # 19
            for k in range(8):
                dve.wait_ge(s_pe, PE_T1(k))
                dve.tensor_copy(r1T[:, k, :], psT[k % 4]).then_inc(s_d, 1)
            for k in range(8):
                dve.wait_ge(s_pe, PE_T2(k))
                dve.tensor_copy(r2T[:, k, :], psT[k % 4]).then_inc(s_d, 1)

    return nc


def _get_nc():
    key = ("nc", WARM)
    if key not in _CACHE:
        _CACHE[key] = _build()
    return _CACHE[key]


# ---------------------------------------------------------------- host prep
def _prep_weights(W1, b1, np_dt):
    """W1 packed by stack layout + Sel matrices."""
    w1np = np.zeros((128, NSTACK, REP), dtype=np.float32)
    selnp = np.zeros((HW, HW * 32), dtype=np.float32)
    for t, ijs in enumerate(SLOTS):
        st, base = t // 4, 32 * (t % 4)
        off = 0
        for ij in ijs:
            i, j = ij // H, ij % H
            for m, kl in enumerate(_klist(ij)):
                k, l = kl // H, kl % H
                ph = (k - i) // 2 + 7
                pw = (l - j) // 2 + 7
                f = (ph * P + pw) * HW + ij
                w1np[base + off + m, st, :] = W1[:, f]
                selnp[kl, ij * 32 + off + m] = 1.0
            off += len(_klist(ij))
    w1np[96, 5, :] = b1
    return w1np.reshape(128, NSTACK * REP).astype(np_dt), selnp.astype(np_dt)


def _pack_x(p1, p2, np_dt):
    # xh[c, ch, t, h, b, ij]
    xt = np.stack([p1, p2], axis=0)  # [t, 128b, h, c, ij]
    xt = xt.reshape(2, NCHUNK, CB, 2, 128, HW).transpose(4, 1, 0, 3, 2, 5)
    return np.ascontiguousarray(xt).reshape(128, NCHUNK * CHW).astype(np_dt)


# ---------------------------------------------------------------- entry
def kernel(patch1, patch2, W1, b1, W2, b2, W3, b3):
    global LAST_EXEC_NS
    import ml_dtypes

    np_dt = ml_dtypes.bfloat16

    patch1 = np.asarray(patch1, dtype=np.float32).reshape(B, 2, 128, HW)
    patch2 = np.asarray(patch2, dtype=np.float32).reshape(B, 2, 128, HW)
    W1 = np.asarray(W1, dtype=np.float32)
    W2 = np.asarray(W2, dtype=np.float32)
    W3 = np.asarray(W3, dtype=np.float32)
    b1 = np.asarray(b1, dtype=np.float32)
    b2 = np.asarray(b2, dtype=np.float32)
    b3 = np.asarray(b3, dtype=np.float32)

    w1e, sele = _prep_weights(W1, b1, np_dt)
    w2e = np.ascontiguousarray(
        W2.T.reshape(8, 128, REP).transpose(1, 0, 2).reshape(128, 8 * REP)
    ).astype(np_dt)
    w3p = np.zeros((8, 128, 16), dtype=np.float32)
    w3p[:, :, 0:4] = W3.T.reshape(8, 128, 4)
    w3e = np.ascontiguousarray(
        w3p.transpose(1, 0, 2).reshape(128, 128)
    ).astype(np_dt)
    b3p = np.zeros((1, 16), dtype=np.float32)
    b3p[0, 0:4] = b3

    shared = {
        "selh": sele,
        "w1h": w1e,
        "w2h": w2e,
        "w3h": w3e,
        "b2h": b2.reshape(1, REP).astype(np_dt),
        "b3h": b3p.astype(np_dt),
        "onesh": np.ones((1, 128), dtype=np_dt),
        "identh": np.eye(128, dtype=np.float32).astype(np_dt),
        "zbh": np.zeros((128, 1), dtype=np.float32),
    }

    in_maps = []
    for i in range(N_CORES):
        sl = slice(i * BL, (i + 1) * BL)
        xh = _pack_x(patch1[sl], patch2[sl], np_dt)
        in_maps.append({"xh": xh, **shared})

    nc = _get_nc()
    trace = os.environ.get("CORR_TRACE", "0") == "1"
    res = run_bass_kernel_spmd(nc, in_maps, list(range(N_CORES)), trace=trace)
    LAST_EXEC_NS = res.exec_time_ns

    out = np.concatenate(
        [res.results[i]["outh"][0:4].T for i in range(N_CORES)], axis=0
    ).astype(np.float32)
    return out


# revision 60
# speedup vs baseline: 1.0449x; 1.0449x over previous
"""CorrelationHead Trainium2 kernel (v3: saturated-DMA schedule + slim sync).

Math: SpatialCorrelationSampler(patch=16, dil=2) on 7x7 maps zero-pads x2 by
(14,16). corr[b] (12544 features) has exactly 2401 distinct nonzero values =
Gram matrix G[b][kl,ij] = sum_c x1[b,c,ij]*x2[b,c,kl], and only parity-valid
(kl,ij) pairs (k=i mod 2, l=j mod 2) ever appear in corr: 625 of 2401.
fc1(corr) = sum over the 625 valid pairs of G * W1eff, so we:
  1. gram:  per-RoI G^T[kl, ij] on PE (K=256 contraction, 2 matmuls),
     evict to SBUF gsa[49, 128b, 49ij] (DVE), 16 chunks of 8 RoIs.
  2. stack: 49 selection matmuls per b-half (0/1 Sel, M=32-row slot, N=64)
     scatter the 625 valid (kl,ij) rows into 6 dense K<=128 "stacks" in PSUM.
     b1 folds in as a constant ones-row. Half 0 is spread over the gram
     stream (3 slots per chunk 8..15); half 1 interleaves with fc1.
  3. fc1:   6 stack-outer matmul pairs (hf inner, stationary reused) paced
     by the per-stack w1 DMAs.
  4. tail:  PE transpose (4-slot PSUM ring) + fc2 (k-outer, w2-DMA-paced)
     + transpose + fc3 in output-transposed form (psO[4,128b], N=128
     streams, 4-descriptor output DMA; host transposes back).

DMA: the two HWDGE queues carry x strictly first (evens on SP, odds on ACT;
~410 GB/s aggregate saturation), then w1 per-stack, then w2 per-k, so the
x stream is never starved by weight traffic. The gpsimd SWDGE carries only
the small constants (sel, biases, identity) up front. An early dummy Relu
pre-loads the ACT activation table off the critical path.

Sync: 6 counting semaphores total (two HWDGE queue counters, one SWDGE
counter, one per compute engine) with cumulative thresholds; HWDGE FIFO
order per engine slice makes >=16*(k+1) waits exact. Fewer semaphores
also shrink the sequencer teardown that is included in measured exec time.

Sharding: pure data-parallel over the 1024 RoIs -> 128 per each of 8 cores.
"""

import os
import numpy as np

import concourse.bass as bass
import concourse.mybir as mybir
from concourse.bass_utils import run_bass_kernel_spmd

# ---------------------------------------------------------------- constants
P = 16
DIL = 2
H = 7
C = 256
B = 1024
REP = 1024
HW = H * H  # 49
N_CORES = 8
BL = B // N_CORES  # 128 RoIs per core

NCHUNK = 16
CB = BL // NCHUNK        # 8 RoIs per chunk
NG = 8                   # RoIs per gram PSUM group (bank = 2KB)
GPC = 1                  # gram groups per chunk
NGROUP = BL // NG        # 16
CHW = 2 * 2 * CB * HW    # els per partition per chunk = 1568

F32 = mybir.dt.float32
BF16 = mybir.dt.bfloat16

WARM = int(os.environ.get("CORR_WARM", "1"))

LAST_EXEC_NS = None
_CACHE = {}


# ------------------------------------------------------------- stack layout
def _klist(ij):
    i, j = ij // H, ij % H
    return [k * H + l for k in range(i % 2, H, 2) for l in range(j % 2, H, 2)]


def _layout():
    """Slot/stack assignment for the 625 valid (kl,ij) pairs.

    Returns slots: list of lists of ij; slot t lives at stack t//4,
    partition base 32*(t%4). Each ij occupies rows [off, off+nkl) of its
    slot where off = sum of nkl of earlier ijs in the slot.
    """
    EE = [i * H + j for i in range(0, H, 2) for j in range(0, H, 2)]  # 16x16
    EO = [i * H + j for i in range(0, H, 2) for j in range(1, H, 2)]  # 12x12
    OE = [i * H + j for i in range(1, H, 2) for j in range(0, H, 2)]  # 12x12
    OO = [i * H + j for i in range(1, H, 2) for j in range(1, H, 2)]  # 9x9
    slots = []
    slots += [[EE[2 * t], EE[2 * t + 1]] for t in range(8)]           # 32 rows
    slots += [[EO[2 * t], EO[2 * t + 1]] for t in range(6)]           # 24 rows
    slots += [[OE[2 * t], OE[2 * t + 1]] for t in range(6)]           # 24 rows
    slots += [[OO[3 * t], OO[3 * t + 1], OO[3 * t + 2]] for t in range(3)]
    assert len(slots) == 23
    return slots


SLOTS = _layout()
NSTACK = 6
STACK_K = [128, 128, 128, 128, 128, 97]  # stack 5: 3 slots + bias row at 96

# ------------------------------------------------- static sync thresholds
# s_pe increment positions (PE program order)
PE_GRAM = lambda gi: gi + 1                  # gram group gi evictable
PE_PERM1 = lambda s: 17 + s                  # perm half-1 stack s complete
PE_F10, PE_F11 = 23, 24                      # fc1 psF0 / psF1 done
PE_T1 = lambda k: 25 + k                     # transpose1 k in psT ring
PE_F20, PE_F21 = 33, 34                      # fc2 psF0 / psF1 done
PE_T2 = lambda k: 35 + k                     # transpose2 k in psT ring
PE_FC3 = 43                                  # fc3 psO done
# s_d increment positions (DVE program order)
D_GRAM = lambda gi: gi + 1                   # gsa group gi landed
D_STK = lambda s: 17 if s < 4 else 19        # Ssb stacks landed (bank copies)
D_CB1 = lambda k: 20 + k                     # r1T k landed
D_CB2 = lambda k: 28 + k                     # r2T k landed
# s_r increment positions (ACT program order)
R_RELU1 = lambda q: 1 + q
R_RELU2 = lambda q: 5 + q
R_OUT = 9
# DMA semaphores: increments from successive DMAs on a queue interleave
# across the 16 SDMA engine slices, so only the TOTAL over all DMAs on a
# semaphore is a valid wait. We pair each even-queue DMA with its odd-queue
# sibling on one semaphore (the pair lands together since the queues stream
# in parallel) and wait for 32.
WC_ALL = 16 * 8  # all gp const DMAs (tiny, land early; single total wait)


# ---------------------------------------------------------------- device IR
def _build():
    dt = BF16
    nc = bass.Bass()

    xh = nc.dram_tensor("xh", [128, NCHUNK * CHW], dt, kind="ExternalInput")
    selh = nc.dram_tensor("selh", [HW, HW * 32], dt, kind="ExternalInput")
    w1h = nc.dram_tensor("w1h", [128, NSTACK * REP], dt, kind="ExternalInput")
    w2h = nc.dram_tensor("w2h", [128, 8 * REP], dt, kind="ExternalInput")
    # fc3 M padded 4 -> 16 (zero cols) so psO[16, 128] is fully written and
    # the output DMA carries 16 partition descriptors (reliable +16 sem inc)
    w3h = nc.dram_tensor("w3h", [128, 8 * 16], dt, kind="ExternalInput")
    b2h = nc.dram_tensor("b2h", [1, REP], dt, kind="ExternalInput")
    b3h = nc.dram_tensor("b3h", [1, 16], dt, kind="ExternalInput")
    onesh = nc.dram_tensor("onesh", [1, 128], dt, kind="ExternalInput")
    identh = nc.dram_tensor("identh", [128, 128], dt, kind="ExternalInput")
    zbh = nc.dram_tensor("zbh", [128, 1], F32, kind="ExternalInput")
    # 16 partitions (rows 4:16 garbage): a HWDGE DMA only nets +16 on its
    # semaphore when all 16 SDMA engine slices carry descriptors
    outh = nc.dram_tensor("outh", [16, 128], F32, kind="ExternalOutput")

    from contextlib import ExitStack

    with ExitStack() as ctx:
        sb = lambda name, shape, d: ctx.enter_context(nc.sbuf_tensor(name, shape, d))
        ps = lambda name, shape, d: ctx.enter_context(nc.psum_tensor(name, shape, d))
        sem = lambda name: ctx.enter_context(nc.semaphore(name))

        xs = sb("xs", [128, NCHUNK, CHW], dt)
        gsa = sb("gsa", [HW, BL, HW], dt)
        sel = sb("sel", [HW, HW * 32], dt)
        w1s = sb("w1s", [128, NSTACK, REP], dt)
        w2s = sb("w2s", [128, 8 * REP], dt)
        w3s = sb("w3s", [128, 8 * 16], dt)
        Ssb = sb("Ssb", [128, NSTACK, 128], dt)
        b2s = sb("b2s", [1, REP], dt)
        b3s = sb("b3s", [1, 16], dt)
        ones = sb("ones", [1, 128], dt)
        idents = sb("idents", [128, 128], dt)
        zbias = sb("zbias", [128, 1], F32)
        dscr = sb("dscr", [128, 65], dt)  # dummy-relu + keepalive scratch
        relu1 = sb("relu1", [128, REP], dt)
        r1T = sb("r1T", [128, 8, 128], dt)
        relu2 = sb("relu2", [128, REP], dt)
        r2T = sb("r2T", [128, 8, 128], dt)
        outs = sb("outs", [16, 128], F32)

        psG0 = ps("psG0", [128, NG * HW], F32)      # 392 f32 = 1568B
        psG1 = ps("psG1", [128, NG * HW], F32)
        psS0 = ps("psS0", [128, 4, 128], F32)
        psS1 = ps("psS1", [128, 384], F32)  # stack4, stack5, psO region
        psF0 = ps("psF0", [128, 512], F32)
        psF1 = ps("psF1", [128, 512], F32)
        psT0 = ps("psT0", [128, 128], dt)
        psT1 = ps("psT1", [128, 128], dt)
        psO = psS1[0:16, 256:384]

        s_x = [sem(f"s_x{p}") for p in range(NCHUNK // 2)]   # chunk pairs
        s_w1 = sem("s_w1")                                   # both w1 halves
        s_w2 = [sem(f"s_w2{r}") for r in range(2)]           # w2 quad pairs
        s_wc = sem("s_wc")
        s_pe = sem("s_pe")
        s_d = sem("s_d")
        s_r = sem("s_r")
        s_o = sem("s_o")
        s_ka = sem("s_ka")  # keepalive junk (1-descriptor DMA, unreliable inc)

        block = ctx.enter_context(nc.Block())
        psG = [psG0, psG1]
        psF = [psF0, psF1]
        # 4-slot transpose ring: 2 dedicated bf16 banks + the (dead by then)
        # gram banks viewed as bf16
        psT = [
            psT0[:, :],
            psT1[:, :],
            psG0[:, 0:64].bitcast(BF16),
            psG1[:, 0:64].bitcast(BF16),
        ]

        # x slice: chunk layout per partition = [t, h, b, ij]
        def xsl(ch, t, h, b):
            off = ((t * 2 + h) * CB + b) * HW
            return xs[:, ch, off : off + HW]

        # ---------------- SP: even x chunks, w1 stacks 0-2, w2 k 0-1 / 4-5
        @block.sync
        def _(sp):
            for ch in range(0, NCHUNK, 2):
                sp.dma_start(
                    xs[:, ch, :], xh[:, ch * CHW : (ch + 1) * CHW]
                ).then_inc(s_x[ch // 2], 16)
            sp.dma_start(w1s[:, 0:3, :], w1h[:, 0 : 3 * REP]).then_inc(s_w1, 16)
            sp.dma_start(w2s[:, 0 : 2 * REP], w2h[:, 0 : 2 * REP]).then_inc(
                s_w2[0], 16
            )
            sp.dma_start(
                w2s[:, 4 * REP : 6 * REP], w2h[:, 4 * REP : 6 * REP]
            ).then_inc(s_w2[1], 16)
            sp.wait_ge(s_r, R_OUT)
            sp.dma_start(outh[:, :], outs[:, :]).then_inc(s_o, 16)
            sp.wait_ge(s_o, 16)

        # ---------------- GPSIMD: small constants via SWDGE
        @block.gpsimd
        def _(gp):
            gp.dma_start(zbias[:, :], zbh[:, :]).then_inc(s_wc, 16)
            gp.dma_start(sel[:, :], selh[:, :]).then_inc(s_wc, 16)
            gp.dma_start(ones[:, :], onesh[:, :]).then_inc(s_wc, 16)
            gp.dma_start(Ssb[96:97, 5, :], onesh[:, :]).then_inc(s_wc, 16)
            gp.dma_start(idents[:, :], identh[:, :]).then_inc(s_wc, 16)
            gp.dma_start(w3s[:, :], w3h[:, :]).then_inc(s_wc, 16)
            gp.dma_start(b2s[:, :], b2h[:, :]).then_inc(s_wc, 16)
            gp.dma_start(b3s[:, :], b3h[:, :]).then_inc(s_wc, 16)

        # ---------------- ACT: odd x chunks, w1 1/3/5, w2 k odd, relus, out
        @block.scalar
        def _(act):
            for ch in range(1, NCHUNK, 2):
                act.dma_start(
                    xs[:, ch, :], xh[:, ch * CHW : (ch + 1) * CHW]
                ).then_inc(s_x[ch // 2], 16)
            act.dma_start(w1s[:, 3:6, :], w1h[:, 3 * REP : 6 * REP]).then_inc(
                s_w1, 16
            )
            act.dma_start(
                w2s[:, 2 * REP : 4 * REP], w2h[:, 2 * REP : 4 * REP]
            ).then_inc(s_w2[0], 16)
            act.dma_start(
                w2s[:, 6 * REP : 8 * REP], w2h[:, 6 * REP : 8 * REP]
            ).then_inc(s_w2[1], 16)

            # pre-load the ACT activation table off the critical path
            act.wait_ge(s_wc, WC_ALL)
            act.activation(
                dscr[:, 0:1], zbias[:, :],
                mybir.ActivationFunctionType.Relu, bias=zbias[:, :],
            )

            for q in range(4):
                act.wait_ge(s_pe, PE_F10 if q < 2 else PE_F11)
                act.activation(
                    relu1[:, q * 256 : (q + 1) * 256],
                    psF[q // 2][:, (q % 2) * 256 : (q % 2) * 256 + 256],
                    mybir.ActivationFunctionType.Relu, bias=zbias[:, :],
                ).then_inc(s_r, 1)
            for q in range(4):
                act.wait_ge(s_pe, PE_F20 if q < 2 else PE_F21)
                act.activation(
                    relu2[:, q * 256 : (q + 1) * 256],
                    psF[q // 2][:, (q % 2) * 256 : (q % 2) * 256 + 256],
                    mybir.ActivationFunctionType.Relu, bias=zbias[:, :],
                ).then_inc(s_r, 1)
            # keepalive on the ACT HWDGE ring so the final output DMA does
            # not pay a cold-ring restart
            act.dma_start(dscr[0:1, 1:65], onesh[0:1, 0:64]).then_inc(s_ka, 16)
            act.wait_ge(s_pe, PE_FC3)
            act.activation(
                outs[:, :], psO, mybir.ActivationFunctionType.Copy
            ).then_inc(s_r, 1)


        # ---------------- PE: all matmuls
        @block.tensor
        def _(pe):
            # HAM warmer: a throwaway N<=512 matmul into psF0 (clobbered by
            # the next start=True accumulation) to keep the PE clock at 8/8
            def warm(n=1, w=512):
                if not WARM:
                    return
                for _ in range(n):
                    pe.matmul(
                        psF0[:, 0:w], xs[:, 0, 0:128], xs[:, 0, 0:w],
                        start=True, stop=True,
                    )

            # stack-scatter matmuls for slot t, batch columns c0:c1
            def perm_slot(t, c0, c1):
                ijs = SLOTS[t]
                st, base = t // 4, 32 * (t % 4)
                for u, ij in enumerate(ijs):
                    if st < 4:
                        pst = psS0[base : base + 32, st, c0:c1]
                    else:
                        pst = psS1[
                            base : base + 32,
                            (st - 4) * 128 + c0 : (st - 4) * 128 + c1,
                        ]
                    mm = pe.matmul(
                        pst,
                        sel[:, ij * 32 : (ij + 1) * 32],
                        gsa[:, c0:c1, ij],
                        start=(u == 0),
                        stop=(u == len(ijs) - 1),
                        tile_position=(0, base),
                    )
                return mm

            # fc1 for one stack: psF0/psF1 accumulate, stationary reused
            def fc1_stack(s):
                pe.wait_ge(s_w1, 32)
                pe.wait_ge(s_d, D_STK(s))
                ks = STACK_K[s]
                for hf in range(2):
                    mm = pe.matmul(
                        psF[hf][:, :],
                        Ssb[0:ks, s, :],
                        w1s[0:ks, s, hf * 512 : hf * 512 + 512],
                        start=(s == 0),
                        stop=(s == NSTACK - 1),
                    )
                    if s == NSTACK - 1:
                        mm.then_inc(s_pe, 1)  # PE_F10 then PE_F11

            # gram: G[b]^T[kl, ij] per RoI; perm half-0 spread over the tail
            for ch in range(NCHUNK):
                if ch % 2 == 0:
                    pe.wait_ge(s_x[ch // 2], 32)
                if ch >= 2:
                    pe.wait_ge(s_d, D_GRAM(ch - 2))
                for bb in range(NG):
                    for h in range(2):
                        mm = pe.matmul(
                            psG[ch % 2][0:HW, bb * HW : (bb + 1) * HW],
                            xsl(ch, 1, h, bb),
                            xsl(ch, 0, h, bb),
                            start=(h == 0),
                            stop=(h == 1),
                        )
                mm.then_inc(s_pe, 1)  # PE_GRAM(ch)
                warm(1, 512)
                # perm half-0 spread over chunks 8..15 (x stream gaps)
                if ch >= 8:
                    if ch == 8:
                        pe.wait_ge(s_d, D_GRAM(7))
                        pe.wait_ge(s_wc, WC_ALL)
                    for t in range(3 * (ch - 8), min(3 * (ch - 8) + 3, 23)):
                        perm_slot(t, 0, 64)

            # perm half-1, then fc1 (w1 lands after perm1 anyway)
            pe.wait_ge(s_d, D_GRAM(NGROUP - 1))
            for s in range(NSTACK):
                for t in range(4 * s, min(4 * s + 4, 23)):
                    mm = perm_slot(t, 64, 128)
                mm.then_inc(s_pe, 1)  # PE_PERM1(s)
                warm(1, 512)
            for s in range(NSTACK):
                fc1_stack(s)

            # transpose relu1 -> psT ring (DVE copies back)
            for k in range(8):
                pe.wait_ge(s_r, R_RELU1(k // 2))
                if k >= 4:
                    pe.wait_ge(s_d, D_CB1(k - 4))
                pe.transpose(
                    psT[k % 4], relu1[:, k * 128 : (k + 1) * 128], idents[:, :]
                ).then_inc(s_pe, 1)  # PE_T1(k)
                if k < 7:
                    warm(1, 256)

            # fc2: bias first (start=True), then 8 K-chunks, w2-DMA-paced,
            # stationary r1T[k] reused across the hf pair
            pe.wait_ge(s_r, R_RELU1(1))
            pe.matmul(psF0[:, :], ones[:, :], b2s[:, 0:512], start=True, stop=False)
            pe.wait_ge(s_r, R_RELU1(3))
            pe.matmul(psF1[:, :], ones[:, :], b2s[:, 512:1024], start=True, stop=False)
            for k in range(8):
                if k % 4 == 0:
                    pe.wait_ge(s_w2[k // 4], 32)
                pe.wait_ge(s_d, D_CB1(k))
                for hf in range(2):
                    mm = pe.matmul(
                        psF[hf][:, :],
                        r1T[:, k, :],
                        w2s[:, k * REP + hf * 512 : k * REP + hf * 512 + 512],
                        start=False,
                        stop=(k == 7),
                    )
                    if k == 7:
                        mm.then_inc(s_pe, 1)  # PE_F20 then PE_F21

            # transpose relu2 -> psT ring
            for k in range(8):
                pe.wait_ge(s_r, R_RELU2(k // 2))
                if k >= 4:
                    pe.wait_ge(s_d, D_CB2(k - 4))
                pe.transpose(
                    psT[k % 4], relu2[:, k * 128 : (k + 1) * 128], idents[:, :]
                ).then_inc(s_pe, 1)  # PE_T2(k)
                if k < 7:
                    warm(1, 256)

            # fc3 transposed: psO[16, 128b] = W3pad @ relu2^T + b3 (rows 4:16
            # are zero-padded so the 16-partition output DMA is fully defined)
            pe.matmul(psO, b3s[0:1, 0:16], ones[0:1, 0:128], start=True, stop=False)
            for k in range(8):
                pe.wait_ge(s_d, D_CB2(k))
                mm = pe.matmul(
                    psO,
                    w3s[:, k * 16 : (k + 1) * 16],
                    r2T[:, k, :],
                    start=False,
                    stop=(k == 7),
                )
            mm.then_inc(s_pe, 1)  # PE_FC3

        # ---------------- DVE: gram evicts, stack evicts, transpose copybacks
        @block.vector
        def _(dve):
            for gi in range(NGROUP):
                dve.wait_ge(s_pe, PE_GRAM(gi))
                dve.tensor_copy(
                    gsa[:, gi * NG : (gi + 1) * NG, :], psG[gi % 2][0:HW, :]
                ).then_inc(s_d, 1)  # D_GRAM(gi)
            # bank-coarse stack evicts (a read may not overlap any open
            # accumulation group in the same PSUM bank)
            dve.wait_ge(s_pe, PE_PERM1(3))
            dve.tensor_copy(Ssb[:, 0:4, :], psS0[:, :, :]).then_inc(s_d, 1)  # 17
            dve.wait_ge(s_pe, PE_PERM1(5))
            dve.tensor_copy(Ssb[:, 4, :], psS1[:, 0:128]).then_inc(s_d, 1)   # 18
            dve.tensor_copy(Ssb[0:96, 5, :], psS1[0:96, 128:256]).then_inc(
                s_d, 1
            )  # 19
            for k in range(8):
                dve.wait_ge(s_pe, PE_T1(k))
                dve.tensor_copy(r1T[:, k, :], psT[k % 4]).then_inc(s_d, 1)
            for k in range(8):
                dve.wait_ge(s_pe, PE_T2(k))
                dve.tensor_copy(r2T[:, k, :], psT[k % 4]).then_inc(s_d, 1)

    return nc


def _get_nc():
    key = ("nc", WARM)
    if key not in _CACHE:
        _CACHE[key] = _build()
    return _CACHE[key]


# ---------------------------------------------------------------- host prep
def _prep_weights(W1, b1, np_dt):
    """W1 packed by stack layout + Sel matrices."""
    w1np = np.zeros((128, NSTACK, REP), dtype=np.float32)
    selnp = np.zeros((HW, HW * 32), dtype=np.float32)
    for t, ijs in enumerate(SLOTS):
        st, base = t // 4, 32 * (t % 4)
        off = 0
        for ij in ijs:
            i, j = ij // H, ij % H
            for m, kl in enumerate(_klist(ij)):
                k, l = kl // H, kl % H
                ph = (k - i) // 2 + 7
                pw = (l - j) // 2 + 7
                f = (ph * P + pw) * HW + ij
                w1np[base + off + m, st, :] = W1[:, f]
                selnp[kl, ij * 32 + off + m] = 1.0
            off += len(_klist(ij))
    w1np[96, 5, :] = b1
    return w1np.reshape(128, NSTACK * REP).astype(np_dt), selnp.astype(np_dt)


def _pack_x(p1, p2, np_dt):
    # xh[c, ch, t, h, b, ij]
    xt = np.stack([p1, p2], axis=0)  # [t, 128b, h, c, ij]
    xt = xt.reshape(2, NCHUNK, CB, 2, 128, HW).transpose(4, 1, 0, 3, 2, 5)
    return np.ascontiguousarray(xt).reshape(128, NCHUNK * CHW).astype(np_dt)


# ---------------------------------------------------------------- entry
def kernel(patch1, patch2, W1, b1, W2, b2, W3, b3):
    global LAST_EXEC_NS
    import ml_dtypes

    np_dt = ml_dtypes.bfloat16

    patch1 = np.asarray(patch1, dtype=np.float32).reshape(B, 2, 128, HW)
    patch2 = np.asarray(patch2, dtype=np.float32).reshape(B, 2, 128, HW)
    W1 = np.asarray(W1, dtype=np.float32)
    W2 = np.asarray(W2, dtype=np.float32)
    W3 = np.asarray(W3, dtype=np.float32)
    b1 = np.asarray(b1, dtype=np.float32)
    b2 = np.asarray(b2, dtype=np.float32)
    b3 = np.asarray(b3, dtype=np.float32)

    w1e, sele = _prep_weights(W1, b1, np_dt)
    w2e = np.ascontiguousarray(
        W2.T.reshape(8, 128, REP).transpose(1, 0, 2).reshape(128, 8 * REP)
    ).astype(np_dt)
    w3p = np.zeros((8, 128, 16), dtype=np.float32)
    w3p[:, :, 0:4] = W3.T.reshape(8, 128, 4)
    w3e = np.ascontiguousarray(
        w3p.transpose(1, 0, 2).reshape(128, 128)
    ).astype(np_dt)
    b3p = np.zeros((1, 16), dtype=np.float32)
    b3p[0, 0:4] = b3

    shared = {
        "selh": sele,
        "w1h": w1e,
        "w2h": w2e,
        "w3h": w3e,
        "b2h": b2.reshape(1, REP).astype(np_dt),
        "b3h": b3p.astype(np_dt),
        "onesh": np.ones((1, 128), dtype=np_dt),
        "identh": np.eye(128, dtype=np.float32).astype(np_dt),
        "zbh": np.zeros((128, 1), dtype=np.float32),
    }

    in_maps = []
    for i in range(N_CORES):
        sl = slice(i * BL, (i + 1) * BL)
        xh = _pack_x(patch1[sl], patch2[sl], np_dt)
        in_maps.append({"xh": xh, **shared})

    nc = _get_nc()
    trace = os.environ.get("CORR_TRACE", "0") == "1"
    res = run_bass_kernel_spmd(nc, in_maps, list(range(N_CORES)), trace=trace)
    LAST_EXEC_NS = res.exec_time_ns

    out = np.concatenate(
        [res.results[i]["outh"][0:4].T for i in range(N_CORES)], axis=0
    ).astype(np.float32)
    return out


# revision 61
# speedup vs baseline: 1.0897x; 1.0429x over previous
"""CorrelationHead Trainium2 kernel (v3: saturated-DMA schedule + slim sync).

Math: SpatialCorrelationSampler(patch=16, dil=2) on 7x7 maps zero-pads x2 by
(14,16). corr[b] (12544 features) has exactly 2401 distinct nonzero values =
Gram matrix G[b][kl,ij] = sum_c x1[b,c,ij]*x2[b,c,kl], and only parity-valid
(kl,ij) pairs (k=i mod 2, l=j mod 2) ever appear in corr: 625 of 2401.
fc1(corr) = sum over the 625 valid pairs of G * W1eff, so we:
  1. gram:  per-RoI G^T[kl, ij] on PE (K=256 contraction, 2 matmuls),
     evict to SBUF gsa[49, 128b, 49ij] (DVE), 16 chunks of 8 RoIs.
  2. stack: 49 selection matmuls per b-half (0/1 Sel, M=32-row slot, N=64)
     scatter the 625 valid (kl,ij) rows into 6 dense K<=128 "stacks" in PSUM.
     b1 folds in as a constant ones-row. Half 0 is spread over the gram
     stream (3 slots per chunk 8..15); half 1 interleaves with fc1.
  3. fc1:   6 stack-outer matmul pairs (hf inner, stationary reused) paced
     by the per-stack w1 DMAs.
  4. tail:  PE transpose (4-slot PSUM ring) + fc2 (k-outer, w2-DMA-paced)
     + transpose + fc3 in output-transposed form (psO[4,128b], N=128
     streams, 4-descriptor output DMA; host transposes back).

DMA: the two HWDGE queues carry x strictly first (evens on SP, odds on ACT;
~410 GB/s aggregate saturation), then w1 per-stack, then w2 per-k, so the
x stream is never starved by weight traffic. The gpsimd SWDGE carries only
the small constants (sel, biases, identity) up front. An early dummy Relu
pre-loads the ACT activation table off the critical path.

Sync: 6 counting semaphores total (two HWDGE queue counters, one SWDGE
counter, one per compute engine) with cumulative thresholds; HWDGE FIFO
order per engine slice makes >=16*(k+1) waits exact. Fewer semaphores
also shrink the sequencer teardown that is included in measured exec time.

Sharding: pure data-parallel over the 1024 RoIs -> 128 per each of 8 cores.
"""

import os
import numpy as np

import concourse.bass as bass
import concourse.mybir as mybir
from concourse.bass_utils import run_bass_kernel_spmd

# ---------------------------------------------------------------- constants
P = 16
DIL = 2
H = 7
C = 256
B = 1024
REP = 1024
HW = H * H  # 49
N_CORES = 8
BL = B // N_CORES  # 128 RoIs per core

NCHUNK = 16
CB = BL // NCHUNK        # 8 RoIs per chunk
NG = 8                   # RoIs per gram PSUM group (bank = 2KB)
GPC = 1                  # gram groups per chunk
NGROUP = BL // NG        # 16
CHW = 2 * 2 * CB * HW    # els per partition per chunk = 1568

F32 = mybir.dt.float32
BF16 = mybir.dt.bfloat16

WARM = int(os.environ.get("CORR_WARM", "1"))

LAST_EXEC_NS = None
_CACHE = {}


# ------------------------------------------------------------- stack layout
def _klist(ij):
    i, j = ij // H, ij % H
    return [k * H + l for k in range(i % 2, H, 2) for l in range(j % 2, H, 2)]


def _layout():
    """Slot/stack assignment for the 625 valid (kl,ij) pairs.

    Returns slots: list of lists of ij; slot t lives at stack t//4,
    partition base 32*(t%4). Each ij occupies rows [off, off+nkl) of its
    slot where off = sum of nkl of earlier ijs in the slot.
    """
    EE = [i * H + j for i in range(0, H, 2) for j in range(0, H, 2)]  # 16x16
    EO = [i * H + j for i in range(0, H, 2) for j in range(1, H, 2)]  # 12x12
    OE = [i * H + j for i in range(1, H, 2) for j in range(0, H, 2)]  # 12x12
    OO = [i * H + j for i in range(1, H, 2) for j in range(1, H, 2)]  # 9x9
    slots = []
    slots += [[EE[2 * t], EE[2 * t + 1]] for t in range(8)]           # 32 rows
    slots += [[EO[2 * t], EO[2 * t + 1]] for t in range(6)]           # 24 rows
    slots += [[OE[2 * t], OE[2 * t + 1]] for t in range(6)]           # 24 rows
    slots += [[OO[3 * t], OO[3 * t + 1], OO[3 * t + 2]] for t in range(3)]
    assert len(slots) == 23
    return slots


SLOTS = _layout()
NSTACK = 6
STACK_K = [128, 128, 128, 128, 128, 97]  # stack 5: 3 slots + bias row at 96

# ------------------------------------------------- static sync thresholds
# s_pe increment positions (PE program order)
PE_GRAM = lambda gi: gi + 1                  # gram group gi evictable
PE_PERM1 = lambda s: 17 + s                  # perm half-1 stack s complete
PE_F10, PE_F11 = 23, 24                      # fc1 psF0 / psF1 done
PE_T1 = lambda k: 25 + k                     # transpose1 k in psT ring
PE_F20, PE_F21 = 33, 34                      # fc2 psF0 / psF1 done
PE_T2 = lambda k: 35 + k                     # transpose2 k in psT ring
PE_FC3 = 43                                  # fc3 psO done
# s_d increment positions (DVE program order)
D_GRAM = lambda gi: gi + 1                   # gsa group gi landed
D_STK = lambda s: 17 if s < 4 else 19        # Ssb stacks landed (bank copies)
D_CB1 = lambda k: 20 + k                     # r1T k landed
D_CB2 = lambda k: 28 + k                     # r2T k landed
# s_r increment positions (ACT program order)
R_RELU1 = lambda q: 1 + q
R_RELU2 = lambda q: 5 + q
R_OUT = 9
# DMA semaphores: increments from successive DMAs on a queue interleave
# across the 16 SDMA engine slices, so only the TOTAL over all DMAs on a
# semaphore is a valid wait. We pair each even-queue DMA with its odd-queue
# sibling on one semaphore (the pair lands together since the queues stream
# in parallel) and wait for 32.
WC_ALL = 16 * 8  # all gp const DMAs (tiny, land early; single total wait)


# ---------------------------------------------------------------- device IR
def _build():
    dt = BF16
    nc = bass.Bass()

    xh = nc.dram_tensor("xh", [128, NCHUNK * CHW], dt, kind="ExternalInput")
    selh = nc.dram_tensor("selh", [HW, HW * 32], dt, kind="ExternalInput")
    w1h = nc.dram_tensor("w1h", [128, NSTACK * REP], dt, kind="ExternalInput")
    w2h = nc.dram_tensor("w2h", [128, 8 * REP], dt, kind="ExternalInput")
    # fc3 M padded 4 -> 16 (zero cols) so psO[16, 128] is fully written and
    # the output DMA carries 16 partition descriptors (reliable +16 sem inc)
    w3h = nc.dram_tensor("w3h", [128, 8 * 16], dt, kind="ExternalInput")
    b2h = nc.dram_tensor("b2h", [1, REP], dt, kind="ExternalInput")
    b3h = nc.dram_tensor("b3h", [1, 16], dt, kind="ExternalInput")
    onesh = nc.dram_tensor("onesh", [1, 128], dt, kind="ExternalInput")
    identh = nc.dram_tensor("identh", [128, 128], dt, kind="ExternalInput")
    zbh = nc.dram_tensor("zbh", [128, 1], F32, kind="ExternalInput")
    # 16 partitions (rows 4:16 garbage): a HWDGE DMA only nets +16 on its
    # semaphore when all 16 SDMA engine slices carry descriptors
    outh = nc.dram_tensor("outh", [16, 128], F32, kind="ExternalOutput")

    from contextlib import ExitStack

    with ExitStack() as ctx:
        sb = lambda name, shape, d: ctx.enter_context(nc.sbuf_tensor(name, shape, d))
        ps = lambda name, shape, d: ctx.enter_context(nc.psum_tensor(name, shape, d))
        sem = lambda name: ctx.enter_context(nc.semaphore(name))

        xs = sb("xs", [128, NCHUNK, CHW], dt)
        gsa = sb("gsa", [HW, BL, HW], dt)
        sel = sb("sel", [HW, HW * 32], dt)
        w1s = sb("w1s", [128, NSTACK, REP], dt)
        w2s = sb("w2s", [128, 8 * REP], dt)
        w3s = sb("w3s", [128, 8 * 16], dt)
        Ssb = sb("Ssb", [128, NSTACK, 128], dt)
        b2s = sb("b2s", [1, REP], dt)
        b3s = sb("b3s", [1, 16], dt)
        ones = sb("ones", [1, 128], dt)
        idents = sb("idents", [128, 128], dt)
        zbias = sb("zbias", [128, 1], F32)
        dscr = sb("dscr", [128, 65], dt)  # dummy-relu + keepalive scratch
        relu1 = sb("relu1", [128, REP], dt)
        r1T = sb("r1T", [128, 8, 128], dt)
        relu2 = sb("relu2", [128, REP], dt)
        r2T = sb("r2T", [128, 8, 128], dt)
        outs = sb("outs", [16, 128], F32)

        psG0 = ps("psG0", [128, NG * HW], F32)      # 392 f32 = 1568B
        psG1 = ps("psG1", [128, NG * HW], F32)
        psS0 = ps("psS0", [128, 4, 128], F32)
        psS1 = ps("psS1", [128, 384], F32)  # stack4, stack5, psO region
        psF0 = ps("psF0", [128, 512], F32)
        psF1 = ps("psF1", [128, 512], F32)
        psT0 = ps("psT0", [128, 128], dt)
        psT1 = ps("psT1", [128, 128], dt)
        psO = psS1[0:16, 256:384]

        s_x = [sem(f"s_x{p}") for p in range(NCHUNK // 2)]   # chunk pairs
        s_w1 = sem("s_w1")                                   # both w1 halves
        s_w2 = [sem(f"s_w2{r}") for r in range(2)]           # w2 quad pairs
        s_wc = sem("s_wc")
        s_pe = sem("s_pe")
        s_d = sem("s_d")
        s_r = sem("s_r")
        s_o = sem("s_o")
        s_ka = sem("s_ka")  # keepalive junk (1-descriptor DMA, unreliable inc)

        block = ctx.enter_context(nc.Block())
        psG = [psG0, psG1]
        psF = [psF0, psF1]
        # 4-slot transpose ring: 2 dedicated bf16 banks + the (dead by then)
        # gram banks viewed as bf16
        psT = [
            psT0[:, :],
            psT1[:, :],
            psG0[:, 0:64].bitcast(BF16),
            psG1[:, 0:64].bitcast(BF16),
        ]

        # x slice: chunk layout per partition = [t, h, b, ij]
        def xsl(ch, t, h, b):
            off = ((t * 2 + h) * CB + b) * HW
            return xs[:, ch, off : off + HW]

        # ---------------- SP: even x chunks, w1 stacks 0-2, w2 k 0-1 / 4-5
        @block.sync
        def _(sp):
            for ch in range(0, NCHUNK, 2):
                sp.dma_start(
                    xs[:, ch, :], xh[:, ch * CHW : (ch + 1) * CHW]
                ).then_inc(s_x[ch // 2], 16)
            sp.dma_start(w1s[:, 0:3, :], w1h[:, 0 : 3 * REP]).then_inc(s_w1, 16)
            sp.dma_start(w2s[:, 0 : 2 * REP], w2h[:, 0 : 2 * REP]).then_inc(
                s_w2[0], 16
            )
            sp.dma_start(
                w2s[:, 4 * REP : 6 * REP], w2h[:, 4 * REP : 6 * REP]
            ).then_inc(s_w2[1], 16)
            sp.wait_ge(s_r, R_OUT)
            sp.dma_start(outh[:, :], outs[:, :]).then_inc(s_o, 16)
            sp.wait_ge(s_o, 16)

        # ---------------- GPSIMD: small constants via SWDGE
        @block.gpsimd
        def _(gp):
            gp.dma_start(zbias[:, :], zbh[:, :]).then_inc(s_wc, 16)
            gp.dma_start(sel[:, :], selh[:, :]).then_inc(s_wc, 16)
            gp.dma_start(ones[:, :], onesh[:, :]).then_inc(s_wc, 16)
            gp.dma_start(Ssb[96:97, 5, :], onesh[:, :]).then_inc(s_wc, 16)
            gp.dma_start(idents[:, :], identh[:, :]).then_inc(s_wc, 16)
            gp.dma_start(w3s[:, :], w3h[:, :]).then_inc(s_wc, 16)
            gp.dma_start(b2s[:, :], b2h[:, :]).then_inc(s_wc, 16)
            gp.dma_start(b3s[:, :], b3h[:, :]).then_inc(s_wc, 16)

        # ---------------- ACT: odd x chunks, w1 1/3/5, w2 k odd, relus, out
        @block.scalar
        def _(act):
            for ch in range(1, NCHUNK, 2):
                act.dma_start(
                    xs[:, ch, :], xh[:, ch * CHW : (ch + 1) * CHW]
                ).then_inc(s_x[ch // 2], 16)
            act.dma_start(w1s[:, 3:6, :], w1h[:, 3 * REP : 6 * REP]).then_inc(
                s_w1, 16
            )
            act.dma_start(
                w2s[:, 2 * REP : 4 * REP], w2h[:, 2 * REP : 4 * REP]
            ).then_inc(s_w2[0], 16)
            act.dma_start(
                w2s[:, 6 * REP : 8 * REP], w2h[:, 6 * REP : 8 * REP]
            ).then_inc(s_w2[1], 16)

            # pre-load the ACT activation table off the critical path
            act.wait_ge(s_wc, WC_ALL)
            act.activation(
                dscr[:, 0:1], zbias[:, :],
                mybir.ActivationFunctionType.Relu, bias=zbias[:, :],
            )

            for q in range(4):
                act.wait_ge(s_pe, PE_F10 if q < 2 else PE_F11)
                act.activation(
                    relu1[:, q * 256 : (q + 1) * 256],
                    psF[q // 2][:, (q % 2) * 256 : (q % 2) * 256 + 256],
                    mybir.ActivationFunctionType.Relu, bias=zbias[:, :],
                ).then_inc(s_r, 1)
            for q in range(4):
                act.wait_ge(s_pe, PE_F20 if q < 2 else PE_F21)
                act.activation(
                    relu2[:, q * 256 : (q + 1) * 256],
                    psF[q // 2][:, (q % 2) * 256 : (q % 2) * 256 + 256],
                    mybir.ActivationFunctionType.Relu, bias=zbias[:, :],
                ).then_inc(s_r, 1)
            # keepalive on the ACT HWDGE ring so the final output DMA does
            # not pay a cold-ring restart
            act.dma_start(dscr[0:1, 1:65], onesh[0:1, 0:64]).then_inc(s_ka, 16)
            act.wait_ge(s_pe, PE_FC3)
            act.activation(
                outs[:, :], psO, mybir.ActivationFunctionType.Copy
            ).then_inc(s_r, 1)


        # ---------------- PE: all matmuls
        @block.tensor
        def _(pe):
            # HAM warmer: a throwaway N<=512 matmul into psF0 (clobbered by
            # the next start=True accumulation) to keep the PE clock at 8/8
            def warm(n=1, w=512):
                if not WARM:
                    return
                for _ in range(n):
                    pe.matmul(
                        psF0[:, 0:w], xs[:, 0, 0:128], xs[:, 0, 0:w],
                        start=True, stop=True,
                    )

            # stack-scatter matmuls for slot t, batch columns c0:c1
            def perm_slot(t, c0, c1):
                ijs = SLOTS[t]
                st, base = t // 4, 32 * (t % 4)
                for u, ij in enumerate(ijs):
                    if st < 4:
                        pst = psS0[base : base + 32, st, c0:c1]
                    else:
                        pst = psS1[
                            base : base + 32,
                            (st - 4) * 128 + c0 : (st - 4) * 128 + c1,
                        ]
                    mm = pe.matmul(
                        pst,
                        sel[:, ij * 32 : (ij + 1) * 32],
                        gsa[:, c0:c1, ij],
                        start=(u == 0),
                        stop=(u == len(ijs) - 1),
                        tile_position=(0, base),
                    )
                return mm

            # fc1 for one stack: psF0/psF1 accumulate, stationary reused
            def fc1_stack(s):
                pe.wait_ge(s_w1, 32)
                pe.wait_ge(s_d, D_STK(s))
                ks = STACK_K[s]
                for hf in range(2):
                    mm = pe.matmul(
                        psF[hf][:, :],
                        Ssb[0:ks, s, :],
                        w1s[0:ks, s, hf * 512 : hf * 512 + 512],
                        start=(s == 0),
                        stop=(s == NSTACK - 1),
                    )
                    if s == NSTACK - 1:
                        mm.then_inc(s_pe, 1)  # PE_F10 then PE_F11

            # gram: G[b]^T[kl, ij] per RoI; perm half-0 spread over the tail
            for ch in range(NCHUNK):
                if ch % 2 == 0:
                    pe.wait_ge(s_x[ch // 2], 32)
                if ch >= 2:
                    pe.wait_ge(s_d, D_GRAM(ch - 2))
                for bb in range(NG):
                    for h in range(2):
                        mm = pe.matmul(
                            psG[ch % 2][0:HW, bb * HW : (bb + 1) * HW],
                            xsl(ch, 1, h, bb),
                            xsl(ch, 0, h, bb),
                            start=(h == 0),
                            stop=(h == 1),
                        )
                mm.then_inc(s_pe, 1)  # PE_GRAM(ch)
                warm(1, 512)
                # perm half-0 spread over chunks 8..15 (x stream gaps)
                if ch >= 8:
                    if ch == 8:
                        pe.wait_ge(s_d, D_GRAM(7))
                        pe.wait_ge(s_wc, WC_ALL)
                    for t in range(3 * (ch - 8), min(3 * (ch - 8) + 3, 23)):
                        perm_slot(t, 0, 64)

            # perm half-1, then fc1 (w1 lands after perm1 anyway)
            pe.wait_ge(s_d, D_GRAM(NGROUP - 1))
            for s in range(NSTACK):
                for t in range(4 * s, min(4 * s + 4, 23)):
                    mm = perm_slot(t, 64, 128)
                mm.then_inc(s_pe, 1)  # PE_PERM1(s)
            for s in range(NSTACK):
                fc1_stack(s)

            # transpose relu1 -> psT ring (DVE copies back)
            for k in range(8):
                pe.wait_ge(s_r, R_RELU1(k // 2))
                if k >= 4:
                    pe.wait_ge(s_d, D_CB1(k - 4))
                pe.transpose(
                    psT[k % 4], relu1[:, k * 128 : (k + 1) * 128], idents[:, :]
                ).then_inc(s_pe, 1)  # PE_T1(k)
                if k < 7:
                    warm(1, 256)

            # fc2: bias first (start=True), then 8 K-chunks, w2-DMA-paced,
            # stationary r1T[k] reused across the hf pair
            pe.wait_ge(s_r, R_RELU1(1))
            pe.matmul(psF0[:, :], ones[:, :], b2s[:, 0:512], start=True, stop=False)
            pe.wait_ge(s_r, R_RELU1(3))
            pe.matmul(psF1[:, :], ones[:, :], b2s[:, 512:1024], start=True, stop=False)
            for k in range(8):
                if k % 4 == 0:
                    pe.wait_ge(s_w2[k // 4], 32)
                pe.wait_ge(s_d, D_CB1(k))
                for hf in range(2):
                    mm = pe.matmul(
                        psF[hf][:, :],
                        r1T[:, k, :],
                        w2s[:, k * REP + hf * 512 : k * REP + hf * 512 + 512],
                        start=False,
                        stop=(k == 7),
                    )
                    if k == 7:
                        mm.then_inc(s_pe, 1)  # PE_F20 then PE_F21

            # transpose relu2 -> psT ring
            for k in range(8):
                pe.wait_ge(s_r, R_RELU2(k // 2))
                if k >= 4:
                    pe.wait_ge(s_d, D_CB2(k - 4))
                pe.transpose(
                    psT[k % 4], relu2[:, k * 128 : (k + 1) * 128], idents[:, :]
                ).then_inc(s_pe, 1)  # PE_T2(k)

            # fc3 transposed: psO[16, 128b] = W3pad @ relu2^T + b3 (rows 4:16
            # are zero-padded so the 16-partition output DMA is fully defined)
            pe.matmul(psO, b3s[0:1, 0:16], ones[0:1, 0:128], start=True, stop=False)
            for k in range(8):
                pe.wait_ge(s_d, D_CB2(k))
                mm = pe.matmul(
                    psO,
                    w3s[:, k * 16 : (k + 1) * 16],
                    r2T[:, k, :],
                    start=False,
                    stop=(k == 7),
                )
            mm.then_inc(s_pe, 1)  # PE_FC3

        # ---------------- DVE: gram evicts, stack evicts, transpose copybacks
        @block.vector
        def _(dve):
            for gi in range(NGROUP):
                dve.wait_ge(s_pe, PE_GRAM(gi))
                dve.tensor_copy(
                    gsa[:, gi * NG : (gi + 1) * NG, :], psG[gi % 2][0:HW, :]
                ).then_inc(s_d, 1)  # D_GRAM(gi)
            # bank-coarse stack evicts (a read may not overlap any open
            # accumulation group in the same PSUM bank)
            dve.wait_ge(s_pe, PE_PERM1(3))
            dve.tensor_copy(Ssb[:, 0:4, :], psS0[:, :, :]).then_inc(s_d, 1)  # 17
            dve.wait_ge(s_pe, PE_PERM1(5))
            dve.tensor_copy(Ssb[:, 4, :], psS1[:, 0:128]).then_inc(s_d, 1)   # 18
            dve.tensor_copy(Ssb[0:96, 5, :], psS1[0:96, 128:256]).then_inc(
                s_d, 1
            )  # 19
            for k in range(8):
                dve.wait_ge(s_pe, PE_T1(k))
                dve.tensor_copy(r1T[:, k, :], psT[k % 4]).then_inc(s_d, 1)
            for k in range(8):
                dve.wait_ge(s_pe, PE_T2(k))
                dve.tensor_copy(r2T[:, k, :], psT[k % 4]).then_inc(s_d, 1)

    return nc


def _get_nc():
    key = ("nc", WARM)
    if key not in _CACHE:
        _CACHE[key] = _build()
    return _CACHE[key]


# ---------------------------------------------------------------- host prep
def _prep_weights(W1, b1, np_dt):
    """W1 packed by stack layout + Sel matrices."""
    w1np = np.zeros((128, NSTACK, REP), dtype=np.float32)
    selnp = np.zeros((HW, HW * 32), dtype=np.float32)
    for t, ijs in enumerate(SLOTS):
        st, base = t // 4, 32 * (t % 4)
        off = 0
        for ij in ijs:
            i, j = ij // H, ij % H
            for m, kl in enumerate(_klist(ij)):
                k, l = kl // H, kl % H
                ph = (k - i) // 2 + 7
                pw = (l - j) // 2 + 7
                f = (ph * P + pw) * HW + ij
                w1np[base + off + m, st, :] = W1[:, f]
                selnp[kl, ij * 32 + off + m] = 1.0
            off += len(_klist(ij))
    w1np[96, 5, :] = b1
    return w1np.reshape(128, NSTACK * REP).astype(np_dt), selnp.astype(np_dt)


def _pack_x(p1, p2, np_dt):
    # xh[c, ch, t, h, b, ij]
    xt = np.stack([p1, p2], axis=0)  # [t, 128b, h, c, ij]
    xt = xt.reshape(2, NCHUNK, CB, 2, 128, HW).transpose(4, 1, 0, 3, 2, 5)
    return np.ascontiguousarray(xt).reshape(128, NCHUNK * CHW).astype(np_dt)


# ---------------------------------------------------------------- entry
def kernel(patch1, patch2, W1, b1, W2, b2, W3, b3):
    global LAST_EXEC_NS
    import ml_dtypes

    np_dt = ml_dtypes.bfloat16

    patch1 = np.asarray(patch1, dtype=np.float32).reshape(B, 2, 128, HW)
    patch2 = np.asarray(patch2, dtype=np.float32).reshape(B, 2, 128, HW)
    W1 = np.asarray(W1, dtype=np.float32)
    W2 = np.asarray(W2, dtype=np.float32)
    W3 = np.asarray(W3, dtype=np.float32)
    b1 = np.asarray(b1, dtype=np.float32)
    b2 = np.asarray(b2, dtype=np.float32)
    b3 = np.asarray(b3, dtype=np.float32)

    w1e, sele = _prep_weights(W1, b1, np_dt)
    w2e = np.ascontiguousarray(
        W2.T.reshape(8, 128, REP).transpose(1, 0, 2).reshape(128, 8 * REP)
    ).astype(np_dt)
    w3p = np.zeros((8, 128, 16), dtype=np.float32)
    w3p[:, :, 0:4] = W3.T.reshape(8, 128, 4)
    w3e = np.ascontiguousarray(
        w3p.transpose(1, 0, 2).reshape(128, 128)
    ).astype(np_dt)
    b3p = np.zeros((1, 16), dtype=np.float32)
    b3p[0, 0:4] = b3

    shared = {
        "selh": sele,
        "w1h": w1e,
        "w2h": w2e,
        "w3h": w3e,
        "b2h": b2.reshape(1, REP).astype(np_dt),
        "b3h": b3p.astype(np_dt),
        "onesh": np.ones((1, 128), dtype=np_dt),
        "identh": np.eye(128, dtype=np.float32).astype(np_dt),
        "zbh": np.zeros((128, 1), dtype=np.float32),
    }

    in_maps = []
    for i in range(N_CORES):
        sl = slice(i * BL, (i + 1) * BL)
        xh = _pack_x(patch1[sl], patch2[sl], np_dt)
        in_maps.append({"xh": xh, **shared})

    nc = _get_nc()
    trace = os.environ.get("CORR_TRACE", "0") == "1"
    res = run_bass_kernel_spmd(nc, in_maps, list(range(N_CORES)), trace=trace)
    LAST_EXEC_NS = res.exec_time_ns

    out = np.concatenate(
        [res.results[i]["outh"][0:4].T for i in range(N_CORES)], axis=0
    ).astype(np.float32)
    return out
